# revision 23
# baseline (speedup 1.0000x reference)
"""GCN layer kernel for Trainium2, 8-core SPMD.

Computes: out = (A @ (X @ W + b)) / colsum(A)[:, None],  A = (adj != 0)
with N=8192 nodes, F_in=F_out=512, across 8 NeuronCores.

Sharding (v4): row-shard adjacency and node features (1024 rows per
core), replicate W/b. Each core computes H = X@W+b for its own rows
plus the next LR-1 ranks' rows, all-gathers H (one 1MB tile-major
buffer per rank), and contracts A@H in a per-core rotated tile order
whose rotation is baked into the HOST-side A layout, so the kernel
uses only static slices. The first LR*8 iterations use local H; the
rest read peer blocks from the gathered buffer via 5 batched 1MB DMAs.

Measured-on-hw design notes:
- dma_start costs ~0.65us of issuing-engine time, so everything is
  batched: A arrives as 8 rotated tile-major 2MB bf16 chunks, X/W as
  tile-major 1MB/0.5MB blocks, H-gather reads as whole rank blocks.
- A is binarized and cast to bf16 on the host: the PE runs bf16 lhsT
  at ~215ns/MM vs ~262 for fp8, and the on-device DVE binarize
  (~122G elem/s) would gate the main loop.
- The collective firmware does not begin the first mesh until ~70-90us
  after kernel start regardless of trigger time; LR=3 local blocks
  plus tiny trailing doorbell collectives keep the PE fed and un-gate
  pending meshes as early as the hardware allows.
- Degree: a DVE pass (binarize-accum into per-tile column sums) runs
  decoupled from the PE; its tiny AllGather is third in the CC queue
  and lands long before the final normalization.
- The last A chunk is processed m-outer so the 8 PSUM banks finish
  staggered and evacuation overlaps the final matmuls.
"""
import numpy as np
import ml_dtypes

N = 8192
F = 512
N_CORES = 8
NB = N // N_CORES          # 1024 rows per core
KT = N // 128              # 64 contraction tiles
MT = NB // 128             # 8 output row tiles per core
FI_T = F // 128            # 4 feat-in tiles
CT = 8                     # A chunks (8 tiles each)
LR = 5                     # ranks whose H we compute locally

_cached = {}


def _build():
    import concourse.bacc as bacc
    import concourse.bass as bass
    import concourse.tile as tile
    from concourse import mybir

    f32 = mybir.dt.float32
    bf16 = mybir.dt.bfloat16

    nc = bacc.Bacc("TRN2", target_bir_lowering=False, debug=False,
                   num_devices=N_CORES)
    # rotated tile-major slab: at[p, i*1024+e] = A[blk_k+e, kt(i)*128+p]
    # with kt(i) = (k*8+i) % 64  (baked on host)
    at = nc.dram_tensor("at", [128, KT * NB], bf16, kind="ExternalInput").ap()
    # xt[p, (rr*4+ki)*1024+e] = X[blk_{(k+rr)%8}+e, ki*128+p]
    xt = nc.dram_tensor("xt", [128, LR * FI_T * NB], bf16,
                        kind="ExternalInput").ap()
    w = nc.dram_tensor("w", [128, FI_T * F], bf16, kind="ExternalInput").ap()
    bvec = nc.dram_tensor("bvec", [128, F], bf16, kind="ExternalInput").ap()
    # out[p, m*512+f] = result row 1024*k + 128*m + p
    out = nc.dram_tensor("out", [128, MT * F], f32, kind="ExternalOutput").ap()

    pid = nc.partition_id()

    with tile.TileContext(nc) as tc:
        with tc.tile_pool(name="dram", bufs=1, space="DRAM") as dram, \
             tc.tile_pool(name="p", bufs=1) as p, \
             tc.tile_pool(name="ps", bufs=1, space="PSUM") as ps:
            dum_in = dram.tile([1, 128], f32)
            dum_out = dram.tile([N_CORES, 128], f32, addr_space="Shared")
            dum_in2 = dram.tile([1, 128], f32)
            dum_out2 = dram.tile([N_CORES, 128], f32, addr_space="Shared")
            hg_in = dram.tile([128, MT * F], bf16)
            hg_out = dram.tile([128 * N_CORES, MT * F], bf16,
                               addr_space="Shared")
            dg_in = dram.tile([128, KT], f32)
            dg_out = dram.tile([128 * N_CORES, KT], f32, addr_space="Shared")

            dums = p.tile([1, 128], f32)
            nc.vector.memset(dums[:], 1.0)

            ones1 = p.tile([1, 128], bf16)
            nc.vector.memset(ones1[:], 1.0)
            onesk = p.tile([128, NB], bf16)
            nc.vector.memset(onesk[:], 1.0)
            cs = p.tile([128, KT], f32)

            # ---- small inputs ----
            b_sb = p.tile([128, F], bf16)
            nc.scalar.dma_start(b_sb[:], bvec)
            xts = []
            for rr in range(LR):
                xr = p.tile([128, FI_T * NB], bf16, tag="xt", bufs=LR,
                            name=f"xt{rr}")
                nc.scalar.dma_start(
                    xr[:], xt[:, rr * FI_T * NB:(rr + 1) * FI_T * NB])
                xts.append(xr)
            w_sb = p.tile([128, FI_T * F], bf16)
            nc.sync.dma_start(w_sb[:], w)

            # ---- A stream: 8 rotated 2MB chunks ----
            a_ch = []
            for c in range(CT):
                a_c = p.tile([128, MT * NB], bf16, tag="ach", bufs=5,
                             name=f"ach{c}")
                nc.scalar.dma_start(
                    a_c[:], at[:, c * MT * NB:(c + 1) * MT * NB])
                a_ch.append(a_c)

            def a_sl(i, m):
                # lhsT [128, 128] for iteration i, out tile m
                c, t = i // MT, i % MT
                off = t * NB + m * 128
                return a_ch[c][:, off:off + 128]

            # ---- local H for ranks pid..pid+LR-1 ----
            pms = []
            for m in range(MT):
                pm = ps.tile([128, F], f32, tag=f"pm{m}", name=f"pm{m}",
                             bufs=1)
                pms.append(pm)
            h_loc = []
            for rr in range(LR):
                hcat = p.tile([128, MT * F], bf16, tag="hcat", bufs=LR,
                              name=f"hcat{rr}")
                for nt in range(MT):
                    hp = pms[nt % 4]
                    late = rr >= 3
                    for ki in range(FI_T):
                        nc.tensor.matmul(
                            hp[:],
                            xts[rr][:, ki * NB + nt * 128:
                                    ki * NB + nt * 128 + 128],
                            w_sb[:, ki * F:(ki + 1) * F],
                            start=(ki == 0),
                            stop=(late and ki == FI_T - 1))
                    if late:
                        # bias folds into a DVE evacuation add; only for
                        # the last H blocks so the added DVE work cannot
                        # push the degree chain past the kernel tail
                        nc.vector.tensor_tensor(
                            hcat[:, nt * F:(nt + 1) * F], hp[:], b_sb[:],
                            mybir.AluOpType.add)
                    else:
                        nc.tensor.matmul(hp[:], ones1[:], b_sb[0:1, :],
                                         start=False, stop=True)
                        nc.scalar.copy(hcat[:, nt * F:(nt + 1) * F], hp[:])
                h_loc.append(hcat)
                if rr == 0:
                    # all-gather own hidden, issued as early as possible
                    # (the mesh begins ~40-70us after its trigger, so
                    # every us of trigger lead time counts)
                    nc.gpsimd.dma_start(hg_in[:], hcat[:])
                    nc.gpsimd.collective_compute(
                        "AllGather", mybir.AluOpType.bypass,
                        replica_groups=[list(range(N_CORES))],
                        ins=[hg_in.opt()], outs=[hg_out.opt()],
                    )

            # ---- batched peer H reads (rank pid+LR .. pid+7) ----
            # on the SYNC queue: these block on the AllGather semaphore,
            # and a blocked queue head starves everything behind it —
            # sync only carries late-kernel work (dg_in, out stores).
            h_rem = {}
            for rr in range(LR, N_CORES):
                hr = p.tile([128, MT * F], bf16, tag="hrem",
                            bufs=N_CORES - LR, name=f"hrem{rr}")
                nc.sync.dma_start(
                    hr[:],
                    hg_out[bass.ds(((pid + rr) % N_CORES) * 128, 128), :])
                h_rem[rr] = hr

            def rhs_sl(i):
                rr, nt = i // MT, i % MT
                src = h_loc[rr] if rr < LR else h_rem[rr]
                return src[:, nt * F:(nt + 1) * F]

            # ---- degree pass on DVE (decoupled from PE) ----
            for i in range(KT):
                c, t = i // MT, i % MT
                scr = p.tile([128, NB], bf16, tag="scr", bufs=2,
                             name=f"scr{i}")
                nc.vector.scalar_tensor_tensor(
                    scr[:], a_ch[c][:, t * NB:(t + 1) * NB], 0.0, onesk[:],
                    mybir.AluOpType.not_equal, mybir.AluOpType.mult,
                    accum_out=cs[:, i:i + 1])
            nc.sync.dma_start(dg_in[:], cs[:])
            nc.gpsimd.collective_compute(
                "AllGather", mybir.AluOpType.bypass,
                replica_groups=[list(range(N_CORES))],
                ins=[dg_in.opt()], outs=[dg_out.opt()],
            )

            # ---- main loop ----
            for i in range(KT - MT):
                rhs = rhs_sl(i)
                for m in range(MT):
                    nc.tensor.matmul(pms[m][:], a_sl(i, m), rhs,
                                     start=(i == 0), stop=False)

            # ---- degree combine + reciprocal (overlaps main loop) ----
            # cs is in iteration order: rank r stores kt=(r*8+i)%64 at
            # column i, so our block (kt=pid*8+m) sits at columns
            # [((pid-r)%8)*8, +8) of rank r's slab.
            deg = p.tile([128, MT], f32)
            prt0 = p.tile([128, MT], f32, tag="prt", bufs=4, name="prt0")
            nc.gpsimd.dma_start(prt0[:], dg_out[0:128, bass.ts(pid, MT)])
            nc.vector.tensor_copy(deg[:], prt0[:])
            for r in range(1, N_CORES):
                col = ((pid + (N_CORES - r)) % N_CORES) * MT
                prt = p.tile([128, MT], f32, tag="prt", bufs=4,
                             name=f"prt{r}")
                nc.gpsimd.dma_start(
                    prt[:], dg_out[r * 128:(r + 1) * 128, bass.ds(col, MT)])
                nc.vector.tensor_tensor(deg[:], deg[:], prt[:],
                                        mybir.AluOpType.add)
            rdeg = p.tile([128, MT], f32)
            nc.vector.reciprocal(rdeg[:], deg[:])

            # ---- last chunk m-outer: banks finish staggered ----
            for m in range(MT):
                for i in range(KT - MT, KT):
                    nc.tensor.matmul(pms[m][:], a_sl(i, m), rhs_sl(i),
                                     start=False, stop=(i == KT - 1))
                o_sb = p.tile([128, F], f32, tag="osb", bufs=2,
                              name=f"osb{m}")
                if m % 2 == 0:
                    nc.vector.tensor_scalar(o_sb[:], pms[m][:],
                                            rdeg[:, m:m + 1], None,
                                            mybir.AluOpType.mult)
                else:
                    nc.scalar.mul(o_sb[:], pms[m][:], rdeg[:, m:m + 1])
                q = nc.sync if m % 2 == 0 else nc.gpsimd
                q.dma_start(out[:, m * F:(m + 1) * F], o_sb[:])

    nc.compile()
    return nc


def _get_nc():
    if "nc" not in _cached:
        _cached["nc"] = _build()
    return _cached["nc"]


def kernel(input_features, adj, W, b):
    from concourse.bass_utils import run_bass_kernel_spmd

    bf = ml_dtypes.bfloat16

    a = np.asarray(adj, dtype=np.float32)
    abinT = (a.T != 0.0).astype(bf)            # abinT[j, e] = (A[e, j] != 0)
    xTb = np.asarray(input_features, dtype=np.float32).T.astype(bf)
    wtile = np.ascontiguousarray(
        np.asarray(W, dtype=np.float32).astype(bf)
        .reshape(FI_T, 128, F).transpose(1, 0, 2).reshape(128, FI_T * F))
    bv = np.ascontiguousarray(np.broadcast_to(
        np.asarray(b, dtype=np.float32).astype(bf).reshape(1, F), (128, F)))

    def xt_block(kk):
        blk = slice(kk * NB, (kk + 1) * NB)
        return (xTb[:, blk].reshape(FI_T, 128, NB).transpose(1, 0, 2)
                .reshape(128, FI_T * NB))

    nc = _get_nc()
    in_maps = []
    for k in range(N_CORES):
        blk = slice(k * NB, (k + 1) * NB)
        rot = [(k * MT + i) % KT for i in range(KT)]
        slab = np.ascontiguousarray(
            abinT[:, blk].reshape(KT, 128, NB)[rot].transpose(1, 0, 2)
            .reshape(128, KT * NB))
        xtk = np.ascontiguousarray(np.concatenate(
            [xt_block((k + rr) % N_CORES) for rr in range(LR)], axis=1))
        in_maps.append({"at": slab, "xt": xtk, "w": wtile, "bvec": bv})
    res = run_bass_kernel_spmd(nc, in_maps, core_ids=list(range(N_CORES)))
    blocks = []
    for k in range(N_CORES):
        o = res.results[k]["out"]
        blocks.append(np.transpose(o.reshape(128, MT, F), (1, 0, 2))
                      .reshape(NB, F))
    return np.concatenate(blocks, axis=0)


# revision 24
# speedup vs baseline: 1.2183x; 1.2183x over previous
"""GCN layer kernel for Trainium2, 8-core SPMD.

Computes: out = (A @ (X @ W + b)) / colsum(A)[:, None],  A = (adj != 0)
with N=8192 nodes, F_in=F_out=512, across 8 NeuronCores.

Sharding (v4): row-shard adjacency and node features (1024 rows per
core), replicate W/b. Each core computes H = X@W+b for its own rows
plus the next LR-1 ranks' rows, all-gathers H (one 1MB tile-major
buffer per rank), and contracts A@H in a per-core rotated tile order
whose rotation is baked into the HOST-side A layout, so the kernel
uses only static slices. The first LR*8 iterations use local H; the
rest read peer blocks from the gathered buffer via 5 batched 1MB DMAs.

Measured-on-hw design notes:
- dma_start costs ~0.65us of issuing-engine time, so everything is
  batched: A arrives as 8 rotated tile-major 2MB bf16 chunks, X/W as
  tile-major 1MB/0.5MB blocks, H-gather reads as whole rank blocks.
- A is binarized and cast to bf16 on the host: the PE runs bf16 lhsT
  at ~215ns/MM vs ~262 for fp8, and the on-device DVE binarize
  (~122G elem/s) would gate the main loop.
- The collective firmware does not begin the first mesh until ~70-90us
  after kernel start regardless of trigger time; LR=3 local blocks
  plus tiny trailing doorbell collectives keep the PE fed and un-gate
  pending meshes as early as the hardware allows.
- Degree: a DVE pass (binarize-accum into per-tile column sums) runs
  decoupled from the PE; its tiny AllGather is third in the CC queue
  and lands long before the final normalization.
- The last A chunk is processed m-outer so the 8 PSUM banks finish
  staggered and evacuation overlaps the final matmuls.
"""
import numpy as np
import ml_dtypes

N = 8192
F = 512
N_CORES = 8
NB = N // N_CORES          # 1024 rows per core
KT = N // 128              # 64 contraction tiles
MT = NB // 128             # 8 output row tiles per core
FI_T = F // 128            # 4 feat-in tiles
CT = 8                     # A chunks (8 tiles each)
LR = 5                     # ranks whose H we compute locally

_cached = {}


def _build():
    import concourse.bacc as bacc
    import concourse.bass as bass
    import concourse.tile as tile
    from concourse import mybir

    f32 = mybir.dt.float32
    bf16 = mybir.dt.bfloat16

    nc = bacc.Bacc("TRN2", target_bir_lowering=False, debug=False,
                   num_devices=N_CORES)
    # rotated tile-major slab: at[p, i*1024+e] = A[blk_k+e, kt(i)*128+p]
    # with kt(i) = (k*8+i) % 64  (baked on host)
    at = nc.dram_tensor("at", [128, KT * NB], bf16, kind="ExternalInput").ap()
    # xt[p, (rr*4+ki)*1024+e] = X[blk_{(k+rr)%8}+e, ki*128+p]
    xt = nc.dram_tensor("xt", [128, LR * FI_T * NB], bf16,
                        kind="ExternalInput").ap()
    w = nc.dram_tensor("w", [128, FI_T * F], bf16, kind="ExternalInput").ap()
    bvec = nc.dram_tensor("bvec", [1, F], bf16, kind="ExternalInput").ap()
    # out[p, m*512+f] = result row 1024*k + 128*m + p
    out = nc.dram_tensor("out", [128, MT * F], f32, kind="ExternalOutput").ap()

    pid = nc.partition_id()

    with tile.TileContext(nc) as tc:
        with tc.tile_pool(name="dram", bufs=1, space="DRAM") as dram, \
             tc.tile_pool(name="p", bufs=1) as p, \
             tc.tile_pool(name="ps", bufs=1, space="PSUM") as ps:
            dum_in = dram.tile([1, 128], f32)
            dum_out = dram.tile([N_CORES, 128], f32, addr_space="Shared")
            dum_in2 = dram.tile([1, 128], f32)
            dum_out2 = dram.tile([N_CORES, 128], f32, addr_space="Shared")
            hg_in = dram.tile([128, MT * F], bf16)
            hg_out = dram.tile([128 * N_CORES, MT * F], bf16,
                               addr_space="Shared")
            dg_in = dram.tile([128, KT], f32)
            dg_out = dram.tile([128 * N_CORES, KT], f32, addr_space="Shared")

            dums = p.tile([1, 128], f32)
            nc.vector.memset(dums[:], 1.0)

            ones1 = p.tile([1, 128], bf16)
            nc.vector.memset(ones1[:], 1.0)
            onesk = p.tile([128, NB], bf16)
            nc.vector.memset(onesk[:], 1.0)
            cs = p.tile([128, KT], f32)

            # ---- small inputs ----
            b_sb = p.tile([1, F], bf16)
            nc.scalar.dma_start(b_sb[:], bvec)
            xts = []
            for rr in range(LR):
                xr = p.tile([128, FI_T * NB], bf16, tag="xt", bufs=LR,
                            name=f"xt{rr}")
                nc.scalar.dma_start(
                    xr[:], xt[:, rr * FI_T * NB:(rr + 1) * FI_T * NB])
                xts.append(xr)
            w_sb = p.tile([128, FI_T * F], bf16)
            nc.sync.dma_start(w_sb[:], w)

            # ---- A stream: 8 rotated 2MB chunks ----
            a_ch = []
            for c in range(CT):
                a_c = p.tile([128, MT * NB], bf16, tag="ach", bufs=5,
                             name=f"ach{c}")
                nc.scalar.dma_start(
                    a_c[:], at[:, c * MT * NB:(c + 1) * MT * NB])
                a_ch.append(a_c)

            def a_sl(i, m):
                # lhsT [128, 128] for iteration i, out tile m
                c, t = i // MT, i % MT
                off = t * NB + m * 128
                return a_ch[c][:, off:off + 128]

            # ---- local H for ranks pid..pid+LR-1 ----
            pms = []
            for m in range(MT):
                pm = ps.tile([128, F], f32, tag=f"pm{m}", name=f"pm{m}",
                             bufs=1)
                pms.append(pm)
            h_loc = []
            for rr in range(LR):
                hcat = p.tile([128, MT * F], bf16, tag="hcat", bufs=LR,
                              name=f"hcat{rr}")
                for nt in range(MT):
                    hp = pms[nt % 4]
                    for ki in range(FI_T):
                        nc.tensor.matmul(
                            hp[:],
                            xts[rr][:, ki * NB + nt * 128:
                                    ki * NB + nt * 128 + 128],
                            w_sb[:, ki * F:(ki + 1) * F],
                            start=(ki == 0), stop=False)
                    nc.tensor.matmul(hp[:], ones1[:], b_sb[:],
                                     start=False, stop=True)
                    nc.scalar.copy(hcat[:, nt * F:(nt + 1) * F], hp[:])
                h_loc.append(hcat)
                if rr == 0:
                    # all-gather own hidden, issued as early as possible
                    # (the mesh begins ~40-70us after its trigger, so
                    # every us of trigger lead time counts)
                    nc.gpsimd.dma_start(hg_in[:], hcat[:])
                    nc.gpsimd.collective_compute(
                        "AllGather", mybir.AluOpType.bypass,
                        replica_groups=[list(range(N_CORES))],
                        ins=[hg_in.opt()], outs=[hg_out.opt()],
                    )

            # ---- batched peer H reads (rank pid+LR .. pid+7) ----
            # on the SYNC queue: these block on the AllGather semaphore,
            # and a blocked queue head starves everything behind it —
            # sync only carries late-kernel work (dg_in, out stores).
            h_rem = {}
            for rr in range(LR, N_CORES):
                hr = p.tile([128, MT * F], bf16, tag="hrem",
                            bufs=N_CORES - LR, name=f"hrem{rr}")
                nc.sync.dma_start(
                    hr[:],
                    hg_out[bass.ds(((pid + rr) % N_CORES) * 128, 128), :])
                h_rem[rr] = hr

            def rhs_sl(i):
                rr, nt = i // MT, i % MT
                src = h_loc[rr] if rr < LR else h_rem[rr]
                return src[:, nt * F:(nt + 1) * F]

            # ---- degree pass on DVE (decoupled from PE) ----
            for i in range(KT):
                c, t = i // MT, i % MT
                scr = p.tile([128, NB], bf16, tag="scr", bufs=2,
                             name=f"scr{i}")
                nc.vector.scalar_tensor_tensor(
                    scr[:], a_ch[c][:, t * NB:(t + 1) * NB], 0.0, onesk[:],
                    mybir.AluOpType.not_equal, mybir.AluOpType.mult,
                    accum_out=cs[:, i:i + 1])
            nc.sync.dma_start(dg_in[:], cs[:])
            nc.gpsimd.collective_compute(
                "AllGather", mybir.AluOpType.bypass,
                replica_groups=[list(range(N_CORES))],
                ins=[dg_in.opt()], outs=[dg_out.opt()],
            )

            # ---- main loop ----
            for i in range(KT - MT):
                rhs = rhs_sl(i)
                for m in range(MT):
                    nc.tensor.matmul(pms[m][:], a_sl(i, m), rhs,
                                     start=(i == 0), stop=False)

            # ---- degree combine + reciprocal (overlaps main loop) ----
            # cs is in iteration order: rank r stores kt=(r*8+i)%64 at
            # column i, so our block (kt=pid*8+m) sits at columns
            # [((pid-r)%8)*8, +8) of rank r's slab.
            deg = p.tile([128, MT], f32)
            prt0 = p.tile([128, MT], f32, tag="prt", bufs=4, name="prt0")
            nc.gpsimd.dma_start(prt0[:], dg_out[0:128, bass.ts(pid, MT)])
            nc.vector.tensor_copy(deg[:], prt0[:])
            for r in range(1, N_CORES):
                col = ((pid + (N_CORES - r)) % N_CORES) * MT
                prt = p.tile([128, MT], f32, tag="prt", bufs=4,
                             name=f"prt{r}")
                nc.gpsimd.dma_start(
                    prt[:], dg_out[r * 128:(r + 1) * 128, bass.ds(col, MT)])
                nc.vector.tensor_tensor(deg[:], deg[:], prt[:],
                                        mybir.AluOpType.add)
            rdeg = p.tile([128, MT], f32)
            nc.vector.reciprocal(rdeg[:], deg[:])

            # ---- last chunk m-outer: banks finish staggered ----
            for m in range(MT):
                for i in range(KT - MT, KT):
                    nc.tensor.matmul(pms[m][:], a_sl(i, m), rhs_sl(i),
                                     start=False, stop=(i == KT - 1))
                o_sb = p.tile([128, F], f32, tag="osb", bufs=2,
                              name=f"osb{m}")
                if m % 2 == 0:
                    nc.vector.tensor_scalar(o_sb[:], pms[m][:],
                                            rdeg[:, m:m + 1], None,
                                            mybir.AluOpType.mult)
                else:
                    nc.scalar.mul(o_sb[:], pms[m][:], rdeg[:, m:m + 1])
                q = nc.sync if m % 2 == 0 else nc.gpsimd
                q.dma_start(out[:, m * F:(m + 1) * F], o_sb[:])

    nc.compile()
    return nc


def _get_nc():
    if "nc" not in _cached:
        _cached["nc"] = _build()
    return _cached["nc"]


def kernel(input_features, adj, W, b):
    from concourse.bass_utils import run_bass_kernel_spmd

    bf = ml_dtypes.bfloat16

    a = np.asarray(adj, dtype=np.float32)
    abinT = (a.T != 0.0).astype(bf)            # abinT[j, e] = (A[e, j] != 0)
    xTb = np.asarray(input_features, dtype=np.float32).T.astype(bf)
    wtile = np.ascontiguousarray(
        np.asarray(W, dtype=np.float32).astype(bf)
        .reshape(FI_T, 128, F).transpose(1, 0, 2).reshape(128, FI_T * F))
    bv = np.ascontiguousarray(
        np.asarray(b, dtype=np.float32).astype(bf).reshape(1, F))

    def xt_block(kk):
        blk = slice(kk * NB, (kk + 1) * NB)
        return (xTb[:, blk].reshape(FI_T, 128, NB).transpose(1, 0, 2)
                .reshape(128, FI_T * NB))

    nc = _get_nc()
    in_maps = []
    for k in range(N_CORES):
        blk = slice(k * NB, (k + 1) * NB)
        rot = [(k * MT + i) % KT for i in range(KT)]
        slab = np.ascontiguousarray(
            abinT[:, blk].reshape(KT, 128, NB)[rot].transpose(1, 0, 2)
            .reshape(128, KT * NB))
        xtk = np.ascontiguousarray(np.concatenate(
            [xt_block((k + rr) % N_CORES) for rr in range(LR)], axis=1))
        in_maps.append({"at": slab, "xt": xtk, "w": wtile, "bvec": bv})
    res = run_bass_kernel_spmd(nc, in_maps, core_ids=list(range(N_CORES)))
    blocks = []
    for k in range(N_CORES):
        o = res.results[k]["out"]
        blocks.append(np.transpose(o.reshape(128, MT, F), (1, 0, 2))
                      .reshape(NB, F))
    return np.concatenate(blocks, axis=0)
